# revision 3
# baseline (speedup 1.0000x reference)
"""Multi-head attention (nn_CustomFlashAttention) for 8 Trainium2 NeuronCores.

Sharding (head/tensor parallel per the problem's hint): each of the 8 cores
owns 2 of the 16 heads. Per core inputs:
  - xT : (2048, 4096) fp32  -- x.T (hidden-major), both batches concatenated
  - wq/wk/wv : (2048, 256)  -- that core's 256 rows of the weight, transposed
  - wo : (256, 2048)        -- that core's 256 columns of w_o, transposed
Each core computes q/k/v projections for its heads, full softmax attention,
and its partial output projection out_i = o_i @ w_o_i.T; the host sums the 8
partial outputs (the mathematical all-reduce of the hint).

All matmuls run in float32r (fp32 storage, reduced-precision multiply) which
streams at 1 cycle/row on the PE for moving dims >= 256.
"""

import numpy as np

# ---- problem constants (hardcoded; kernel.py must be self-contained) ----
B = 2          # batch
S = 2048       # sequence length
D = 2048       # hidden dim
NH = 16        # heads
HD = 128       # head dim
NCORES = 8
HPC = NH // NCORES          # heads per core = 2
E = HPC * HD                # per-core projection width = 256
T = B * S                   # total tokens = 4096
KO = D // 128               # contraction tiles over hidden dim = 16
TBLK = 256                  # token block for the projection phase
SCALE = 1.0 / float(np.sqrt(HD))

_CACHE = {}


def _build_nc():
    import concourse.tile as tile
    from concourse import bacc, mybir

    F32, F32R = mybir.dt.float32, mybir.dt.float32r
    Exp = mybir.ActivationFunctionType.Exp

    nc = bacc.Bacc("TRN2", target_bir_lowering=False)
    xT = nc.dram_tensor("xT", [D, T], F32, kind="ExternalInput")
    wq = nc.dram_tensor("wq", [D, E], F32, kind="ExternalInput")
    wk = nc.dram_tensor("wk", [D, E], F32, kind="ExternalInput")
    wv = nc.dram_tensor("wv", [D, E], F32, kind="ExternalInput")
    wo = nc.dram_tensor("wo", [E, D], F32, kind="ExternalInput")
    out = nc.dram_tensor("out", [T, D], F32, kind="ExternalOutput")

    with tile.TileContext(nc) as tc:
        with (
            tc.tile_pool(name="consts", bufs=1) as consts,
            tc.tile_pool(name="batch", bufs=1) as batch,
            tc.tile_pool(name="xs", bufs=2) as xsp,
            tc.tile_pool(name="pt", bufs=4) as ptp,
            tc.tile_pool(name="small", bufs=2) as small,
            tc.tile_pool(name="recb", bufs=2) as recbp,
            tc.tile_pool(name="outp", bufs=4) as outp,
            tc.tile_pool(name="ps", bufs=3, space="PSUM") as ps,
            tc.tile_pool(name="pacc", bufs=2, space="PSUM") as pacc,
            tc.tile_pool(name="psum1", bufs=2, space="PSUM") as psum1,
        ):
            # persistent weights in SBUF, hidden dim on partitions
            wq_sb = consts.tile([128, KO, E], F32R)
            wk_sb = consts.tile([128, KO, E], F32R)
            wv_sb = consts.tile([128, KO, E], F32R)
            wo_sb = consts.tile([128, HPC, D], F32R)
            nc.sync.dma_start(wq_sb[:], wq[:].rearrange("(ko p) e -> p ko e", p=128).bitcast(F32R))
            nc.sync.dma_start(wk_sb[:], wk[:].rearrange("(ko p) e -> p ko e", p=128).bitcast(F32R))
            nc.sync.dma_start(wv_sb[:], wv[:].rearrange("(ko p) e -> p ko e", p=128).bitcast(F32R))
            nc.sync.dma_start(wo_sb[:], wo[:].rearrange("(h p) f -> p h f", p=128).bitcast(F32R))
            ones_f = consts.tile([128, 1], F32)
            nc.vector.memset(ones_f[:], 1.0)
            ones = consts.tile([128, 1], F32R)
            nc.vector.tensor_copy(ones[:], ones_f[:])

            for b in range(B):
                qT = batch.tile([128, HPC, S], F32R, tag="qT")
                kT = batch.tile([128, HPC, S], F32R, tag="kT")
                v = batch.tile([128, S // 128, E], F32R, tag="v")
                oT = batch.tile([128, HPC, S], F32R, tag="oT")

                # ---- phase 1: q/k/v projections for batch b ----
                for tb in range(S // TBLK):
                    t0 = b * S + tb * TBLK
                    xs = xsp.tile([128, KO, TBLK], F32R, tag="xs")
                    nc.sync.dma_start(
                        xs[:],
                        xT[:, t0 : t0 + TBLK].rearrange("(ko p) t -> p ko t", p=128).bitcast(F32R),
                    )
                    for w_sb, dstT in ((wq_sb, qT), (wk_sb, kT)):
                        for eb in range(HPC):
                            pj = ps.tile([128, TBLK], F32, tag="ps")
                            for ko in range(KO):
                                nc.tensor.matmul(
                                    pj[:],
                                    w_sb[:, ko, eb * 128 : (eb + 1) * 128],
                                    xs[:, ko, :],
                                    start=(ko == 0),
                                    stop=(ko == KO - 1),
                                )
                            nc.vector.tensor_copy(
                                dstT[:, eb, tb * TBLK : tb * TBLK + TBLK], pj[:]
                            )
                    for ts in range(TBLK // 128):
                        pj = ps.tile([128, E], F32, tag="ps")
                        for ko in range(KO):
                            nc.tensor.matmul(
                                pj[:],
                                xs[:, ko, ts * 128 : (ts + 1) * 128],
                                wv_sb[:, ko, :],
                                start=(ko == 0),
                                stop=(ko == KO - 1),
                            )
                        nc.vector.tensor_copy(v[:, (tb * TBLK) // 128 + ts, :], pj[:])

                # ---- phase 2: attention per head, scores kept transposed ----
                for h in range(HPC):
                    for tqb in range(S // 512):
                        po = pacc.tile([128, 512], F32, tag="po")
                        sm = psum1.tile([1, 512], F32, tag="sm")
                        for ik in range(S // 128):
                            sc = ps.tile([128, 512], F32, tag="ps")
                            nc.tensor.matmul(
                                sc[:],
                                kT[:, h, ik * 128 : (ik + 1) * 128],
                                qT[:, h, tqb * 512 : (tqb + 1) * 512],
                                start=True,
                                stop=True,
                            )
                            pt = ptp.tile([128, 512], F32R, tag="pt")
                            nc.scalar.activation(pt[:], sc[:], Exp, bias=0.0, scale=SCALE)
                            nc.tensor.matmul(
                                po[:],
                                v[:, ik, h * 128 : (h + 1) * 128],
                                pt[:],
                                start=(ik == 0),
                                stop=(ik == S // 128 - 1),
                            )
                            nc.tensor.matmul(
                                sm[:],
                                ones[:],
                                pt[:],
                                start=(ik == 0),
                                stop=(ik == S // 128 - 1),
                            )
                        rec = small.tile([1, 512], F32, tag="rec")
                        nc.vector.reciprocal(rec[:], sm[:])
                        recb = recbp.tile([128, 512], F32, tag="recb")
                        nc.gpsimd.partition_broadcast(recb[:], rec[0:1, :])
                        nc.vector.tensor_mul(
                            oT[:, h, tqb * 512 : (tqb + 1) * 512], po[:], recb[:]
                        )

                # ---- phase 3: partial output projection ----
                for to in range(S // 128):
                    for fb in range(D // 512):
                        pj = ps.tile([128, 512], F32, tag="ps")
                        for h in range(HPC):
                            nc.tensor.matmul(
                                pj[:],
                                oT[:, h, to * 128 : (to + 1) * 128],
                                wo_sb[:, h, fb * 512 : (fb + 1) * 512],
                                start=(h == 0),
                                stop=(h == HPC - 1),
                            )
                        ot_sb = outp.tile([128, 512], F32, tag="ot_sb")
                        # split PSUM->SBUF copies between DVE and ACT
                        if fb % 2 == 0:
                            nc.vector.tensor_copy(ot_sb[:], pj[:])
                        else:
                            nc.scalar.copy(ot_sb[:], pj[:])
                        nc.sync.dma_start(
                            out[
                                b * S + to * 128 : b * S + (to + 1) * 128,
                                fb * 512 : (fb + 1) * 512,
                            ],
                            ot_sb[:],
                        )
    nc.compile()
    return nc


def _get_nc():
    if "nc" not in _CACHE:
        _CACHE["nc"] = _build_nc()
    return _CACHE["nc"]


def _shard_inputs(x, w_q, w_k, w_v, w_o):
    x = np.asarray(x, dtype=np.float32)
    w_q = np.asarray(w_q, dtype=np.float32)
    w_k = np.asarray(w_k, dtype=np.float32)
    w_v = np.asarray(w_v, dtype=np.float32)
    w_o = np.asarray(w_o, dtype=np.float32)
    xT = np.ascontiguousarray(x.reshape(T, D).T)
    in_maps = []
    for i in range(NCORES):
        e0 = i * E
        in_maps.append(
            {
                "xT": xT,
                "wq": np.ascontiguousarray(w_q[e0 : e0 + E, :].T),
                "wk": np.ascontiguousarray(w_k[e0 : e0 + E, :].T),
                "wv": np.ascontiguousarray(w_v[e0 : e0 + E, :].T),
                "wo": np.ascontiguousarray(w_o[:, e0 : e0 + E].T),
            }
        )
    return in_maps


def run_spmd(x, w_q, w_k, w_v, w_o, **spmd_kwargs):
    """Build+run on cores 0-7; returns (partial results list, BassKernelResults)."""
    from concourse.bass_utils import run_bass_kernel_spmd

    nc = _get_nc()
    in_maps = _shard_inputs(x, w_q, w_k, w_v, w_o)
    res = run_bass_kernel_spmd(nc, in_maps, core_ids=list(range(NCORES)), **spmd_kwargs)
    return res


def kernel(x, w_q, w_k, w_v, w_o):
    res = run_spmd(x, w_q, w_k, w_v, w_o)
    acc = res.results[0]["out"].astype(np.float32)
    for i in range(1, NCORES):
        acc = acc + res.results[i]["out"]
    return acc.reshape(B, S, D)
